# revision 6
# baseline (speedup 1.0000x reference)
"""Trainium2 Bass kernel for nn_ExtendedSelfAttention (B=4, S=2048, D=4096, H=1).

With n_heads=1 the softmax is over a size-1 axis, so attention weights are
exactly 1.0 and the module reduces to:

    out = (value @ Wv.T + bv) @ Wo.T + bo

(query/key/Wq/Wk never affect the output). That is two 8192x4096 @ 4096x4096
GEMMs, data-parallel over the 8192 tokens across 8 NeuronCores.

Per core (1024 tokens):
  phase 1: V^T = w1T.T @ x^T      (lhsT = Wv.T streamed, rhs = x^T resident)
  phase 2: out = (V^T).T @ w2T    (lhsT = V^T resident,  rhs = Wo.T streamed)

The intermediate V^T is produced directly in the layout phase 2 needs for its
stationary operand, so no on-device transposes are required; all transposes
happen on the host (not measured HW time). Compute dtype is bf16 (fp32 PSUM
accumulation), cast on host.
"""

import numpy as np

B, S, D = 4, 2048, 4096
N_CORES = 8
TOK = B * S          # 8192 tokens
TPC = TOK // N_CORES  # 1024 tokens per core
P = 128
KO = D // P          # 32 k-tiles over the contraction dim
TBLK = 512           # phase-1 rhs free dim (token block)
GBLK = 512           # phase-2 rhs free dim (output-feature block)
NT = TPC // TBLK     # 2
NG = D // GBLK       # 8
MT = TPC // P        # 8

_CACHED = {}


def _build_nc():
    import concourse.bass as bass
    import concourse.tile as tile
    from concourse import bacc, mybir

    bf16 = mybir.dt.bfloat16
    f32 = mybir.dt.float32

    nc = bacc.Bacc("TRN2", target_bir_lowering=False, debug=False,
                   num_devices=N_CORES)

    xT = nc.declare_dram_parameter("xT", [P, KO, TPC], bf16, isOutput=False)
    # w1[m, p, ko, c] = Wv[m*128+c, ko*128+p]  (i.e. Wv.T partition-tiled)
    w1 = nc.declare_dram_parameter("w1", [KO, P, KO, P], bf16, isOutput=False)
    # w2[n2, p, ko, c] = Wo[n2*GBLK+c, ko*128+p]
    w2 = nc.declare_dram_parameter("w2", [NG, P, KO, GBLK], bf16, isOutput=False)
    bv = nc.declare_dram_parameter("bv", [P, KO], f32, isOutput=False)
    bo = nc.declare_dram_parameter("bo", [P, D], f32, isOutput=False)
    out = nc.declare_dram_parameter("out", [TPC, D], f32, isOutput=True)

    with tile.TileContext(nc) as tc:
        with tc.tile_pool(name="const", bufs=1) as const_pool, \
             tc.tile_pool(name="vt", bufs=1) as vt_pool, \
             tc.tile_pool(name="psum", bufs=4, space="PSUM") as psum_pool, \
             tc.tile_pool(name="stage", bufs=4) as stage_pool:
            bv_t = const_pool.tile([P, KO], f32)
            nc.sync.dma_start(out=bv_t[:], in_=bv[:])
            bo_t = const_pool.tile([P, D], f32)
            nc.sync.dma_start(out=bo_t[:], in_=bo[:])

            # V^T, partition-tiled: [f-inner, f-outer, t]
            vt = vt_pool.tile([P, KO, TPC], bf16)

            # ---- phase 1: V^T = Wv.T.T @ x^T ----
            with tc.tile_pool(name="xt", bufs=1) as xt_pool, \
                 tc.tile_pool(name="w1p", bufs=3) as w1_pool:
                xt = xt_pool.tile([P, KO, TPC], bf16)
                nc.sync.dma_start(out=xt[:], in_=xT[:])
                for m in range(KO):
                    w1t = w1_pool.tile([P, KO, P], bf16)
                    nc.sync.dma_start(out=w1t[:], in_=w1[m])
                    for n in range(NT):
                        ps = psum_pool.tile([P, TBLK], f32)
                        for k in range(KO):
                            nc.tensor.matmul(
                                ps[:],
                                w1t[:, k, :],
                                xt[:, k, n * TBLK:(n + 1) * TBLK],
                                start=(k == 0),
                                stop=(k == KO - 1),
                            )
                        # evict + bias (per-partition scalar) + cast to bf16
                        nc.vector.tensor_scalar_add(
                            vt[:, m, n * TBLK:(n + 1) * TBLK],
                            ps[:],
                            bv_t[:, m:m + 1],
                        )

            # ---- phase 2: out = (V^T).T @ Wo.T ----
            with tc.tile_pool(name="w2p", bufs=2) as w2_pool:
                for n2 in range(NG):
                    w2t = w2_pool.tile([P, KO, GBLK], bf16)
                    nc.sync.dma_start(out=w2t[:], in_=w2[n2])
                    for m2 in range(MT):
                        ps = psum_pool.tile([P, GBLK], f32)
                        for k in range(KO):
                            nc.tensor.matmul(
                                ps[:],
                                vt[:, k, m2 * P:(m2 + 1) * P],
                                w2t[:, k, :],
                                start=(k == 0),
                                stop=(k == KO - 1),
                            )
                        st = stage_pool.tile([P, GBLK], f32)
                        nc.vector.tensor_tensor(
                            st[:],
                            ps[:],
                            bo_t[:, n2 * GBLK:(n2 + 1) * GBLK],
                            mybir.AluOpType.add,
                        )
                        nc.sync.dma_start(
                            out=out[m2 * P:(m2 + 1) * P,
                                    n2 * GBLK:(n2 + 1) * GBLK],
                            in_=st[:],
                        )
    nc.compile()
    return nc


def _get_nc():
    if "nc" not in _CACHED:
        _CACHED["nc"] = _build_nc()
    return _CACHED["nc"]


def _prep_inputs(value, Wv, bv, Wo, bo):
    import ml_dtypes
    bf16 = ml_dtypes.bfloat16

    x = np.asarray(value, np.float32).reshape(TOK, D)
    # xT_all[p, ko, t] = x[t, ko*128+p]
    xT_all = np.ascontiguousarray(
        x.reshape(TOK, KO, P).transpose(2, 1, 0)).astype(bf16)

    Wv = np.asarray(Wv, np.float32)
    Wo = np.asarray(Wo, np.float32)
    # w1[m, p, ko, c] = Wv[m*128+c, ko*128+p]
    w1 = np.ascontiguousarray(
        Wv.reshape(KO, P, KO, P).transpose(0, 3, 2, 1)).astype(bf16)
    # w2[n2, p, ko, c] = Wo[n2*GBLK+c, ko*128+p]
    w2 = np.ascontiguousarray(
        Wo.reshape(NG, GBLK, KO, P).transpose(0, 3, 2, 1)).astype(bf16)

    bv_p = np.ascontiguousarray(
        np.asarray(bv, np.float32).reshape(KO, P).T)
    bo_p = np.ascontiguousarray(
        np.broadcast_to(np.asarray(bo, np.float32).reshape(1, D), (P, D)))

    in_maps = []
    for c in range(N_CORES):
        in_maps.append({
            "xT": np.ascontiguousarray(xT_all[:, :, c * TPC:(c + 1) * TPC]),
            "w1": w1,
            "w2": w2,
            "bv": bv_p,
            "bo": bo_p,
        })
    return in_maps


def _run(in_maps, trace=False):
    from concourse.bass_utils import run_bass_kernel_spmd
    nc = _get_nc()
    res = run_bass_kernel_spmd(nc, in_maps, list(range(N_CORES)), trace=trace)
    return res


def kernel(**inputs):
    in_maps = _prep_inputs(inputs["value"], inputs["Wv"], inputs["bv"],
                           inputs["Wo"], inputs["bo"])
    res = _run(in_maps, trace=False)
    out = np.empty((TOK, D), np.float32)
    for c in range(N_CORES):
        out[c * TPC:(c + 1) * TPC] = res.results[c]["out"]
    return out.reshape(B, S, D)


# revision 8
# speedup vs baseline: 1.3496x; 1.3496x over previous
"""Trainium2 Bass kernel for nn_ExtendedSelfAttention (B=4, S=2048, D=4096, H=1).

With n_heads=1 the softmax is over a size-1 axis, so attention weights are
exactly 1.0 and the module reduces to:

    out = (value @ Wv.T + bv) @ Wo.T + bo
        = value @ (Wo @ Wv).T + (Wo @ bv + bo)

(query/key/Wq/Wk never affect the output.) Since there are 8192 tokens but
only 4096 features, composing the weights first cuts total FLOPs by 25%:
computing Wc^T = (Wo @ Wv)^T costs one 4096^3 GEMM (sharded 8 ways), after
which only ONE token GEMM is needed instead of two.

Sharding (no collectives):
  phase A: core c computes Wc^T[:, c*512:(c+1)*512]   (1024 matmuls)
           lhsT = Wv[f-tile, k-block] (natural layout), rhs = Wo^T slice
  phase B: core c computes out[:, c*512:(c+1)*512] for ALL 8192 tokens
           lhsT = x^T tiles, rhs = Wc^T slice (SBUF-resident)  (2048 matmuls)
Output is column-sharded; the host concatenates. The fused bias
bias2 = Wo @ bv + bo is computed exactly on the host and added in phase B.

Compute dtype bf16 (host-cast), fp32 PSUM accumulation, fp32 output.
"""

import numpy as np

B, S, D = 4, 2048, 4096
N_CORES = 8
TOK = B * S           # 8192 tokens
P = 128
KO = D // P           # 32 contraction tiles
GBLK = D // N_CORES   # 512 output columns per core
TT = TOK // P         # 64 token tiles

_CACHED = {}


def _build_nc():
    import concourse.bass as bass  # noqa: F401  (registers engine builders)
    import concourse.tile as tile
    from concourse import bacc, mybir

    bf16 = mybir.dt.bfloat16
    f32 = mybir.dt.float32

    nc = bacc.Bacc("TRN2", target_bir_lowering=False, debug=False,
                   num_devices=N_CORES)

    # wv[m, p, fo, c2] = Wv[fo*128+p, m*128+c2]   (lhsT tiles for phase A)
    wv = nc.declare_dram_parameter("wv", [KO, P, KO, P], bf16, isOutput=False)
    # woT[p, fo, g] = Wo[cg0+g, fo*128+p]          (rhs for phase A, per-core)
    woT = nc.declare_dram_parameter("woT", [P, KO, GBLK], bf16, isOutput=False)
    # xt[tt, p, ko, tc] = x[tt*128+tc, ko*128+p]   (lhsT tiles for phase B)
    xt = nc.declare_dram_parameter("xt", [TT, P, KO, P], bf16, isOutput=False)
    b2 = nc.declare_dram_parameter("b2", [P, GBLK], f32, isOutput=False)
    out = nc.declare_dram_parameter("out", [TOK, GBLK], f32, isOutput=True)

    with tile.TileContext(nc) as tc:
        with tc.tile_pool(name="const", bufs=1) as const_pool, \
             tc.tile_pool(name="wot", bufs=1) as wot_pool, \
             tc.tile_pool(name="wct", bufs=1) as wct_pool, \
             tc.tile_pool(name="wvp", bufs=3) as wv_pool, \
             tc.tile_pool(name="xtp", bufs=4) as xt_pool, \
             tc.tile_pool(name="psum", bufs=4, space="PSUM") as psum_pool, \
             tc.tile_pool(name="stage", bufs=4) as stage_pool:
            b2_t = const_pool.tile([P, GBLK], f32)
            nc.sync.dma_start(out=b2_t[:], in_=b2[:])

            wot_sb = wot_pool.tile([P, KO, GBLK], bf16)
            # chunked load so phase A can start after the first chunk
            for fo in range(KO):
                nc.sync.dma_start(out=wot_sb[:, fo, :], in_=woT[:, fo, :])

            wct_sb = wct_pool.tile([P, KO, GBLK], bf16)

            # ---- phase A: Wc^T slice = Wv.T-contracted with Wo^T slice ----
            for mA in range(KO):
                wv_t = wv_pool.tile([P, KO, P], bf16)
                nc.sync.dma_start(out=wv_t[:], in_=wv[mA])
                ps = psum_pool.tile([P, GBLK], f32)
                for fA in range(KO):
                    nc.tensor.matmul(
                        ps[:], wv_t[:, fA, :], wot_sb[:, fA, :],
                        start=(fA == 0), stop=(fA == KO - 1),
                    )
                nc.vector.tensor_copy(wct_sb[:, mA, :], ps[:])

            # ---- phase B: out slice = x @ Wc^T slice (+ bias2) ----
            for tt in range(TT):
                xt_t = xt_pool.tile([P, KO, P], bf16)
                nc.sync.dma_start(out=xt_t[:], in_=xt[tt])
                ps = psum_pool.tile([P, GBLK], f32)
                for k in range(KO):
                    nc.tensor.matmul(
                        ps[:], xt_t[:, k, :], wct_sb[:, k, :],
                        start=(k == 0), stop=(k == KO - 1),
                    )
                st = stage_pool.tile([P, GBLK], f32)
                nc.vector.tensor_add(st[:], ps[:], b2_t[:])
                nc.sync.dma_start(
                    out=out[tt * P:(tt + 1) * P, :], in_=st[:])
    nc.compile()
    return nc


def _get_nc():
    if "nc" not in _CACHED:
        _CACHED["nc"] = _build_nc()
    return _CACHED["nc"]


def _prep_inputs(value, Wv, bv, Wo, bo):
    import ml_dtypes
    bf16 = ml_dtypes.bfloat16

    x = np.asarray(value, np.float32).reshape(TOK, D)
    Wv = np.asarray(Wv, np.float32)
    Wo = np.asarray(Wo, np.float32)
    bv = np.asarray(bv, np.float32)
    bo = np.asarray(bo, np.float32)

    # xt[tt, p, ko, tc] = x[tt*128+tc, ko*128+p]
    xt = np.ascontiguousarray(
        x.reshape(TT, P, KO, P).transpose(0, 3, 2, 1)).astype(bf16)
    # wv_p[m, p, fo, c2] = Wv[fo*128+p, m*128+c2]
    wv_p = np.ascontiguousarray(
        Wv.reshape(KO, P, KO, P).transpose(2, 1, 0, 3)).astype(bf16)
    # woT_full[c][p, fo, g] = Wo[c*GBLK+g, fo*128+p]
    woT_full = Wo.reshape(N_CORES, GBLK, KO, P).transpose(0, 3, 2, 1)

    bias2 = (Wo.astype(np.float64) @ bv.astype(np.float64)
             + bo.astype(np.float64)).astype(np.float32)

    in_maps = []
    for c in range(N_CORES):
        b2_c = np.ascontiguousarray(np.broadcast_to(
            bias2[c * GBLK:(c + 1) * GBLK][None, :], (P, GBLK)))
        in_maps.append({
            "xt": xt,
            "wv": wv_p,
            "woT": np.ascontiguousarray(woT_full[c]).astype(bf16),
            "b2": b2_c,
        })
    return in_maps


def _run(in_maps, trace=False):
    from concourse.bass_utils import run_bass_kernel_spmd
    nc = _get_nc()
    res = run_bass_kernel_spmd(nc, in_maps, list(range(N_CORES)), trace=trace)
    return res


def kernel(**inputs):
    in_maps = _prep_inputs(inputs["value"], inputs["Wv"], inputs["bv"],
                           inputs["Wo"], inputs["bo"])
    res = _run(in_maps, trace=False)
    out = np.empty((TOK, D), np.float32)
    for c in range(N_CORES):
        out[:, c * GBLK:(c + 1) * GBLK] = res.results[c]["out"]
    return out.reshape(B, S, D)
